# revision 1
# baseline (speedup 1.0000x reference)
"""Trainium2 Bass kernel for a 2-layer GAT (nn_GAT_50586124812836).

kernel(**inputs) takes the FULL inputs from reference.setup_inputs() and
returns the full [50000, 32] float32 output. Internally: destination-node
sharding across 8 NeuronCores, edges sorted by dst and padded per 128-dst
tile; per layer a dense phase computes h/alpha per shard, an AllGather
publishes a packed bf16 node table, and an edge phase uses SWDGE dma_gather
(int16 indices, lo/hi table halves) plus one-hot PE matmuls to do the
segment softmax and weighted aggregation entirely on-chip.
"""
import numpy as np
import ml_dtypes

import concourse.mybir as mybir
from concourse import bass
from concourse.bass import AP, MemorySpace
from concourse import ap_utils
from concourse._compat import exact_div


def dma_gather_raw(
    gp,                       # nc.gpsimd
    out_ap: AP,
    in_ap: AP,
    idxs_ap: AP,
    num_idxs: int,
    elem_size: int,
    elem_step: int,
    queue_num: int = 0,
    single_packet: bool = True,
):
    assert idxs_ap.dtype == mybir.dt.int16
    assert in_ap.space == MemorySpace.DRAM
    assert idxs_ap.space == MemorySpace.SBUF
    assert out_ap.space == MemorySpace.SBUF
    assert in_ap.dtype == out_ap.dtype
    dtsz = mybir.dt.size(in_ap.dtype)
    stride_bytes = elem_step * dtsz
    stride_bytes_256 = exact_div(stride_bytes, 256)
    assert 0 < stride_bytes_256 < 256
    assert ap_utils.ap_is_contiguous(in_ap.ap[1:])
    assert ap_utils.ap_is_contiguous(out_ap.ap[1:])
    assert ap_utils.ap_is_contiguous(idxs_ap.ap[1:])
    assert in_ap.ap[0][0] == elem_step
    assert in_ap.ap[-1][1] == elem_size
    assert out_ap.ap[-1][1] == elem_size
    assert num_idxs % 128 == 0
    assert out_ap.ap[0][1] * out_ap.ap[1][1] == num_idxs

    _in_ap = gp.lower_ap_dma(in_ap, for_custom_bir_dma=True)
    _idxs_ap = gp.lower_ap(idxs_ap)
    _out_ap = gp.lower_ap(out_ap)
    inst = gp.add_instruction(
        mybir.InstDMAGatherAnt(
            name=gp.bass.get_next_instruction_name(),
            ins=[
                *_in_ap,
                _idxs_ap,
                gp.lower_val_access(gp.to_reg(num_idxs)),
            ],
            outs=[_out_ap],
            transpose=False,
            num_idxs=num_idxs,
            elem_size=elem_size,
            stride_bytes_256=stride_bytes_256,
            gen_mode=0,
            single_packet=single_packet,
            queue_num=queue_num,
            sbuf_tokens_per_rank=0,
            sbuf_free_dim_per_rank=0,
            sbuf_free_dim_pad_per_rank=0,
            sbuf_byte_offset=0,
        )
    )
    return inst


def wrap_idx16(idx, pad_to=None):
    """Host-side: [n] int array -> [128, ceil(n/16)] int16 wrapped in 16
    partitions (flat i -> partition i%16, slot i//16), replicated to 128."""
    import numpy as np
    idx = np.asarray(idx)
    n = idx.shape[0]
    if pad_to is not None and n < pad_to:
        idx = np.concatenate([idx, np.zeros(pad_to - n, idx.dtype)])
        n = pad_to
    assert n % 16 == 0
    w = idx.reshape(n // 16, 16).T.astype(np.int16)   # [16, n/16]
    return np.ascontiguousarray(np.tile(w, (8, 1)))    # [128, n/16]



import math
from dataclasses import dataclass

import numpy as np
import ml_dtypes

import concourse.bass as bass
import concourse.tile as tile
from concourse import bacc, mybir
from concourse.masks import make_identity
from concourse.library_config import mlp

BF16 = mybir.dt.bfloat16
F32 = mybir.dt.float32
I16 = mybir.dt.int16
P = 128
Alu = mybir.AluOpType
Act = mybir.ActivationFunctionType
NEG_SLOPE = 0.2
BF = ml_dtypes.bfloat16


@dataclass
class Cfg:
    N: int = 50000
    NC: int = 8
    F: int = 512
    H1: int = 8
    HD: int = 8
    D2: int = 32
    CH: int = 10         # chunks per (tile, half)
    TB: int = 7          # tiles per batch

    def __post_init__(self):
        self.D1 = self.H1 * self.HD
        assert self.N % self.NC == 0
        self.SHARD = self.N // self.NC
        self.TILES = math.ceil(self.SHARD / P)
        self.SHARD_PAD = self.TILES * P
        self.V = self.NC * self.SHARD_PAD
        self.VH = self.V // 2
        assert self.VH < 32768
        assert self.TILES % self.TB == 0
        self.NB = self.TILES // self.TB
        assert self.F % P == 0
        self.KC = self.F // P
        self.ROW1 = self.D1 + self.H1           # 72
        self.ROW2 = self.D2 + 1                 # 33
        # gather instruction spans (in tiles) within one batch
        self.SPANS = []
        left = self.TB
        while left > 0:
            s = min(2, left)
            self.SPANS.append(s)
            left -= s
        self.KBH = self.TB * self.CH            # chunks per stream per batch
        self.WCOLS = self.KBH * 8               # wrapped idx cols per batch


def build_program(cfg: Cfg):
    nc = bacc.Bacc("TRN2", target_bir_lowering=False, debug=False,
                   num_devices=cfg.NC)
    dt = nc.dram_tensor
    xT = dt("xT", [cfg.F, cfg.SHARD_PAD], BF16, kind="ExternalInput")
    w1 = dt("w1", [P, cfg.KC * cfg.D1], BF16, kind="ExternalInput")
    w2 = dt("w2", [cfg.D1, cfg.D2], BF16, kind="ExternalInput")
    a1s = dt("a1s", [P, cfg.D1], F32, kind="ExternalInput")
    a1d = dt("a1d", [P, cfg.D1], F32, kind="ExternalInput")
    a2s = dt("a2s", [P, cfg.D2], F32, kind="ExternalInput")
    a2d = dt("a2d", [P, cfg.D2], F32, kind="ExternalInput")
    b1r = dt("b1r", [P, cfg.D1], F32, kind="ExternalInput")
    b2r = dt("b2r", [P, cfg.D2], F32, kind="ExternalInput")
    srcW = {}
    dstW = {}
    dstl = {}
    for s in ("lo", "hi"):
        srcW[s] = dt(f"srcW_{s}", [cfg.NB, P, cfg.WCOLS], I16,
                     kind="ExternalInput")
        dstW[s] = dt(f"dstW_{s}", [cfg.NB, P, cfg.WCOLS], I16,
                     kind="ExternalInput")
        dstl[s] = dt(f"dstl_{s}", [cfg.NB, P, cfg.KBH], BF16,
                     kind="ExternalInput")
    out = dt("out", [cfg.SHARD_PAD, cfg.D2], F32, kind="ExternalOutput")

    gspace = "Shared" if cfg.NC > 4 else "Local"
    ha1_sh = dt("ha1_sh", [cfg.SHARD_PAD, P], BF16, kind="Internal")
    ha1_full = dt("ha1_full", [cfg.V, P], BF16, kind="Internal",
                  addr_space=gspace)
    ad1 = dt("ad1", [cfg.SHARD_PAD, P], BF16, kind="Internal")
    ha2_sh = dt("ha2_sh", [cfg.SHARD_PAD, P], BF16, kind="Internal")
    ha2_full = dt("ha2_full", [cfg.V, P], BF16, kind="Internal",
                  addr_space=gspace)
    ad2 = dt("ad2", [cfg.SHARD_PAD, P], BF16, kind="Internal")

    rg = [list(range(cfg.NC))]

    with tile.TileContext(nc) as tc:
        cpool_cm = tc.tile_pool(name="consts", bufs=1)
        cpool = cpool_cm.__enter__()
        nc.gpsimd.load_library(mlp)
        w1s = cpool.tile([P, cfg.KC, cfg.D1], BF16)
        nc.sync.dma_start(w1s[:], w1[:].rearrange("p (k d) -> p k d", k=cfg.KC))
        w2s = cpool.tile([cfg.D1, cfg.D2], BF16)
        nc.sync.dma_start(w2s[:], w2[:])
        a1s_s = cpool.tile([P, cfg.D1], F32)
        nc.sync.dma_start(a1s_s[:], a1s[:])
        a1d_s = cpool.tile([P, cfg.D1], F32)
        nc.sync.dma_start(a1d_s[:], a1d[:])
        a2s_s = cpool.tile([P, cfg.D2], F32)
        nc.sync.dma_start(a2s_s[:], a2s[:])
        a2d_s = cpool.tile([P, cfg.D2], F32)
        nc.sync.dma_start(a2d_s[:], a2d[:])
        b1_s = cpool.tile([P, cfg.D1], F32)
        nc.sync.dma_start(b1_s[:], b1r[:])
        b2_s = cpool.tile([P, cfg.D2], F32)
        nc.sync.dma_start(b2_s[:], b2r[:])
        iota_i = cpool.tile([P, P], mybir.dt.int32)
        nc.gpsimd.iota(iota_i[:], pattern=[[1, P]], base=0,
                       channel_multiplier=0)
        iota_bf = cpool.tile([P, P], BF16)
        nc.vector.tensor_copy(iota_bf[:], iota_i[:])
        ident = cpool.tile([P, P], BF16)
        make_identity(nc, ident[:])
        ZT = cpool.tile([cfg.D1, cfg.TILES * P], BF16)

        # ---------------- Phase 1: h1 / alpha1 ----------------
        with tc.tile_pool(name="p1", bufs=3) as pool, \
             tc.tile_pool(name="p1ps", bufs=4, space="PSUM") as pps:
            for t in range(cfg.TILES):
                ts = slice(t * P, (t + 1) * P)
                xt = pool.tile([P, cfg.KC, P], BF16, name="xt")
                nc.sync.dma_start(
                    xt[:], xT[:, ts].rearrange("(k p) n -> p k n", p=P))
                h1ps = pps.tile([P, cfg.D1], F32, name="h1ps")
                for k in range(cfg.KC):
                    nc.tensor.matmul(
                        out=h1ps[:], lhsT=xt[:, k, :], rhs=w1s[:, k, :],
                        start=(k == 0), stop=(k == cfg.KC - 1))
                tmp_s = pool.tile([P, cfg.D1], F32, name="tmp_s")
                nc.vector.tensor_tensor(
                    out=tmp_s[:], in0=h1ps[:], in1=a1s_s[:], op=Alu.mult)
                as1 = pool.tile([P, cfg.H1], F32, name="as1")
                nc.vector.tensor_reduce(
                    out=as1[:],
                    in_=tmp_s[:].rearrange("p (h r) -> p h r", h=cfg.H1),
                    axis=mybir.AxisListType.X, op=Alu.add)
                tmp_d = pool.tile([P, cfg.D1], F32, name="tmp_d")
                nc.vector.tensor_tensor(
                    out=tmp_d[:], in0=h1ps[:], in1=a1d_s[:], op=Alu.mult)
                ad1f = pool.tile([P, cfg.H1], F32, name="ad1f")
                nc.vector.tensor_reduce(
                    out=ad1f[:],
                    in_=tmp_d[:].rearrange("p (h r) -> p h r", h=cfg.H1),
                    axis=mybir.AxisListType.X, op=Alu.add)
                ad1b = pool.tile([P, cfg.H1], BF16, name="ad1b")
                nc.vector.tensor_copy(ad1b[:], ad1f[:])
                nc.sync.dma_start(ad1[ts, 0:cfg.H1], ad1b[:])
                ha = pool.tile([P, cfg.ROW1], BF16, name="ha")
                nc.vector.tensor_copy(ha[:, 0:cfg.D1], h1ps[:])
                nc.vector.tensor_copy(ha[:, cfg.D1:cfg.ROW1], as1[:])
                nc.sync.dma_start(ha1_sh[ts, 0:cfg.ROW1], ha[:])

        nc.gpsimd.collective_compute(
            "AllGather", Alu.bypass, replica_groups=rg,
            ins=[ha1_sh[:]], outs=[ha1_full[:]])

        def edge_phase(layer: int):
            if layer == 1:
                ROW, NH, HDv, DV = cfg.ROW1, cfg.H1, cfg.HD, cfg.D1
                Tsrc, Tdst = ha1_full, ad1
            else:
                ROW, NH, HDv, DV = cfg.ROW2, 1, cfg.D2, cfg.D2
                Tsrc, Tdst = ha2_full, ad2
            RH = DV + NH
            halves = {"lo": Tsrc[0:cfg.VH, 0:ROW],
                      "hi": Tsrc[cfg.VH:cfg.V, 0:ROW]}
            with tc.tile_pool(name=f"ep{layer}", bufs=2) as pool, \
                 tc.tile_pool(name=f"ep{layer}ps", bufs=4, space="PSUM") as pps:
                for b in range(cfg.NB):
                    G, EXb, DSTL = {}, {}, {}
                    for s in ("lo", "hi"):
                        iw = pool.tile([P, cfg.WCOLS], I16, name=f"iw{s}")
                        nc.sync.dma_start(iw[:], srcW[s][b])
                        dw = pool.tile([P, cfg.WCOLS], I16, name=f"dw{s}")
                        nc.sync.dma_start(dw[:], dstW[s][b])
                        dl = pool.tile([P, cfg.KBH], BF16, name=f"dl{s}")
                        nc.sync.dma_start(dl[:], dstl[s][b])
                        DSTL[s] = dl
                        g = pool.tile([P, cfg.KBH, ROW], BF16, name=f"G{s}")
                        dgt = pool.tile([P, cfg.KBH, NH], BF16, name=f"Dg{s}")
                        ct = 0
                        for sp in cfg.SPANS:
                            nidx = sp * cfg.CH * P
                            c0, c1 = ct * cfg.CH, (ct + sp) * cfg.CH
                            w0, w1_ = ct * cfg.CH * 8, (ct + sp) * cfg.CH * 8
                            dma_gather_raw(
                                nc.gpsimd, g[:, c0:c1, :], halves[s],
                                iw[:, w0:w1_], nidx, ROW, P,
                                single_packet=False)
                            dma_gather_raw(
                                nc.gpsimd, dgt[:, c0:c1, :], Tdst[:, 0:NH],
                                dw[:, w0:w1_], nidx, NH, P,
                                single_packet=False)
                            ct += sp
                        G[s] = g
                        TE = pool.tile([P, cfg.KBH, NH], F32, name=f"TE{s}")
                        nc.vector.tensor_tensor(
                            out=TE[:], in0=g[:, :, DV:DV + NH], in1=dgt[:],
                            op=Alu.add)
                        TEm = pool.tile([P, cfg.KBH, NH], F32, name=f"TEm{s}")
                        nc.vector.tensor_scalar_mul(TEm[:], TE[:], NEG_SLOPE)
                        LR = pool.tile([P, cfg.KBH, NH], F32, name=f"LR{s}")
                        nc.vector.tensor_tensor(
                            out=LR[:], in0=TE[:], in1=TEm[:], op=Alu.max)
                        ex = pool.tile([P, cfg.KBH, NH], F32, name=f"EXf{s}")
                        nc.scalar.activation(ex[:], LR[:], Act.Exp)
                        exb = pool.tile([P, cfg.KBH, NH], BF16, name=f"EXb{s}")
                        nc.vector.tensor_copy(exb[:], ex[:])
                        EXb[s] = exb
                    for tt in range(cfg.TB):
                        t = b * cfg.TB + tt
                        ts = slice(t * P, (t + 1) * P)
                        cs = slice(tt * cfg.CH, (tt + 1) * cfg.CH)
                        ps = pps.tile([P, RH], F32, name="ps")
                        for si, s in enumerate(("lo", "hi")):
                            oh = pool.tile([P, cfg.CH, P], BF16,
                                           name=f"oh{s}")
                            nc.vector.tensor_tensor(
                                out=oh[:],
                                in0=DSTL[s][:, cs].unsqueeze(2).broadcast_to(
                                    [P, cfg.CH, P]),
                                in1=iota_bf[:].unsqueeze(1).broadcast_to(
                                    [P, cfg.CH, P]),
                                op=Alu.is_equal)
                            R = pool.tile([P, cfg.CH, RH], BF16, name=f"R{s}")
                            nc.vector.tensor_tensor(
                                out=R[:, :, 0:DV].rearrange(
                                    "p c (h r) -> p c h r", h=NH),
                                in0=G[s][:, cs, 0:DV].rearrange(
                                    "p c (h r) -> p c h r", h=NH),
                                in1=EXb[s][:, cs, :].unsqueeze(3).broadcast_to(
                                    [P, cfg.CH, NH, HDv]),
                                op=Alu.mult)
                            nc.vector.tensor_copy(
                                R[:, :, DV:RH], EXb[s][:, cs, :])
                            for c in range(cfg.CH):
                                nc.tensor.matmul(
                                    out=ps[:], lhsT=oh[:, c, :],
                                    rhs=R[:, c, :],
                                    start=(si == 0 and c == 0),
                                    stop=(si == 1 and c == cfg.CH - 1))
                        Se = pool.tile([P, NH], F32, name="Se")
                        nc.vector.tensor_scalar_add(Se[:], ps[:, DV:RH], 1e-30)
                        RS = pool.tile([P, NH], F32, name="RS")
                        nc.vector.reciprocal(RS[:], Se[:])
                        zb = pool.tile([P, DV], F32, name="zb")
                        nc.vector.tensor_tensor(
                            out=zb[:].rearrange("p (h r) -> p h r", h=NH),
                            in0=ps[:, 0:DV].rearrange("p (h r) -> p h r", h=NH),
                            in1=RS[:].unsqueeze(2).broadcast_to([P, NH, HDv]),
                            op=Alu.mult)
                        if layer == 1:
                            zc = pool.tile([P, DV], F32, name="zc")
                            nc.vector.tensor_tensor(
                                out=zc[:], in0=zb[:], in1=b1_s[:], op=Alu.add)
                            mn = pool.tile([P, DV], F32, name="mn")
                            nc.vector.tensor_scalar_min(mn[:], zc[:], 0.0)
                            em = pool.tile([P, DV], F32, name="em")
                            nc.scalar.activation(em[:], mn[:], Act.Exp)
                            rp = pool.tile([P, DV], F32, name="rp")
                            nc.vector.tensor_scalar_max(rp[:], zc[:], 0.0)
                            s1 = pool.tile([P, DV], F32, name="s1")
                            nc.vector.tensor_tensor(
                                out=s1[:], in0=rp[:], in1=em[:], op=Alu.add)
                            zel = pool.tile([P, DV], BF16, name="zel")
                            nc.vector.tensor_scalar_add(zel[:], s1[:], -1.0)
                            ztp = pps.tile([cfg.D1, P], BF16, name="ztp")
                            nc.tensor.transpose(ztp[:], zel[:], ident[:])
                            nc.vector.tensor_copy(ZT[:, ts], ztp[:])
                        else:
                            o2 = pool.tile([P, DV], F32, name="o2")
                            nc.vector.tensor_tensor(
                                out=o2[:], in0=zb[:], in1=b2_s[:], op=Alu.add)
                            nc.sync.dma_start(out[ts, :], o2[:])

        edge_phase(1)

        # ---------------- Phase 4: h2 / alpha2 ----------------
        with tc.tile_pool(name="p4", bufs=3) as pool, \
             tc.tile_pool(name="p4ps", bufs=4, space="PSUM") as pps:
            for t in range(cfg.TILES):
                ts = slice(t * P, (t + 1) * P)
                h2ps = pps.tile([P, cfg.D2], F32, name="h2ps")
                nc.tensor.matmul(
                    out=h2ps[:], lhsT=ZT[:, ts], rhs=w2s[:],
                    start=True, stop=True)
                t2s = pool.tile([P, cfg.D2], F32, name="t2s")
                nc.vector.tensor_tensor(
                    out=t2s[:], in0=h2ps[:], in1=a2s_s[:], op=Alu.mult)
                as2 = pool.tile([P, 1], F32, name="as2")
                nc.vector.tensor_reduce(
                    out=as2[:], in_=t2s[:], axis=mybir.AxisListType.X,
                    op=Alu.add)
                t2d = pool.tile([P, cfg.D2], F32, name="t2d")
                nc.vector.tensor_tensor(
                    out=t2d[:], in0=h2ps[:], in1=a2d_s[:], op=Alu.mult)
                ad2f = pool.tile([P, 1], F32, name="ad2f")
                nc.vector.tensor_reduce(
                    out=ad2f[:], in_=t2d[:], axis=mybir.AxisListType.X,
                    op=Alu.add)
                ad2b = pool.tile([P, 1], BF16, name="ad2b")
                nc.vector.tensor_copy(ad2b[:], ad2f[:])
                nc.sync.dma_start(ad2[ts, 0:1], ad2b[:])
                ha2 = pool.tile([P, cfg.ROW2], BF16, name="ha2")
                nc.vector.tensor_copy(ha2[:, 0:cfg.D2], h2ps[:])
                nc.vector.tensor_copy(ha2[:, cfg.D2:cfg.ROW2], as2[:])
                nc.sync.dma_start(ha2_sh[ts, 0:cfg.ROW2], ha2[:])

        nc.gpsimd.collective_compute(
            "AllGather", Alu.bypass, replica_groups=rg,
            ins=[ha2_sh[:]], outs=[ha2_full[:]])

        edge_phase(2)
        cpool_cm.__exit__(None, None, None)

    nc.compile()
    return nc


# ---------------- host-side preprocessing ----------------

def _wrap16(idx):
    n = idx.shape[0]
    w = idx.reshape(n // 16, 16).T.astype(np.int16)
    return np.tile(w, (8, 1))                      # [128, n/16]


def preprocess_edges(edge_index: np.ndarray, cfg: Cfg):
    N = cfg.N
    src = np.concatenate([np.asarray(edge_index[0]).astype(np.int64),
                          np.arange(N, dtype=np.int64)])
    dst = np.concatenate([np.asarray(edge_index[1]).astype(np.int64),
                          np.arange(N, dtype=np.int64)])
    src_remap = (src // cfg.SHARD) * cfg.SHARD_PAD + (src % cfg.SHARD)
    half = (src_remap >= cfg.VH).astype(np.int64)
    core = dst // cfg.SHARD
    loc = dst % cfg.SHARD
    tl = loc // P
    # group edges by (core, tile, half), order by src for locality
    gid = (core * cfg.TILES + tl) * 2 + half
    order = np.lexsort((src_remap, gid))
    gid, src_remap, loc = gid[order], src_remap[order], loc[order]
    counts = np.bincount(gid, minlength=cfg.NC * cfg.TILES * 2)
    assert counts.max() <= cfg.CH * P, (counts.max(), cfg.CH * P)
    starts = np.zeros(len(counts) + 1, dtype=np.int64)
    np.cumsum(counts, out=starts[1:])
    pos = np.arange(len(gid)) - starts[gid]

    CHP = cfg.CH * P
    shape = (cfg.NC, cfg.TILES, 2, CHP)
    src_pad = np.zeros(shape, dtype=np.int32)
    dloc_pad = np.zeros(shape, dtype=np.int32)
    dstl_pad = np.full(shape, P, dtype=np.float32)
    c_ = gid // (cfg.TILES * 2)
    t_ = (gid // 2) % cfg.TILES
    h_ = gid % 2
    src_pad[c_, t_, h_, pos] = (src_remap - h_ * cfg.VH).astype(np.int32)
    dloc_pad[c_, t_, h_, pos] = loc.astype(np.int32)
    dstl_pad[c_, t_, h_, pos] = (loc % P).astype(np.float32)

    outs = {}
    for hi, s in enumerate(("lo", "hi")):
        sW = np.zeros((cfg.NC, cfg.NB, P, cfg.WCOLS), dtype=np.int16)
        dW = np.zeros((cfg.NC, cfg.NB, P, cfg.WCOLS), dtype=np.int16)
        dL = np.zeros((cfg.NC, cfg.NB, P, cfg.KBH), dtype=np.float32)
        for c in range(cfg.NC):
            for b in range(cfg.NB):
                tt0 = b * cfg.TB
                # wrapped idx, concatenated per instruction span
                col = 0
                ct = 0
                for sp in cfg.SPANS:
                    sv = src_pad[c, tt0 + ct:tt0 + ct + sp, hi].ravel()
                    dv = dloc_pad[c, tt0 + ct:tt0 + ct + sp, hi].ravel()
                    w = sv.shape[0] // 16
                    sW[c, b, :, col:col + w] = _wrap16(sv)
                    dW[c, b, :, col:col + w] = _wrap16(dv)
                    col += w
                    ct += sp
                # dstl in chunk-major lanes: [TB, CH, P] -> [P, TB*CH]
                dl = dstl_pad[c, tt0:tt0 + cfg.TB, hi].reshape(
                    cfg.TB * cfg.CH, P).T
                dL[c, b] = dl
        outs[s] = (sW, dW, dL.astype(BF))
    return outs


def make_in_maps(inputs: dict, cfg: Cfg):
    x = np.asarray(inputs["x"], dtype=np.float32)
    ei = np.asarray(inputs["edge_index"]).astype(np.int64)
    W1 = np.asarray(inputs["W1"], dtype=np.float32)
    a1_src = np.asarray(inputs["a1_src"], dtype=np.float32)
    a1_dst = np.asarray(inputs["a1_dst"], dtype=np.float32)
    b1 = np.asarray(inputs["b1"], dtype=np.float32)
    W2 = np.asarray(inputs["W2"], dtype=np.float32)
    a2_src = np.asarray(inputs["a2_src"], dtype=np.float32)
    a2_dst = np.asarray(inputs["a2_dst"], dtype=np.float32)
    b2 = np.asarray(inputs["b2"], dtype=np.float32)

    ed = preprocess_edges(ei, cfg)
    w1_dev = np.ascontiguousarray(
        W1.reshape(cfg.KC, P, cfg.D1).transpose(1, 0, 2)
        .reshape(P, cfg.KC * cfg.D1)).astype(BF)
    consts = {
        "w1": w1_dev, "w2": W2.astype(BF),
        "a1s": np.broadcast_to(a1_src.reshape(1, cfg.D1), (P, cfg.D1)).copy(),
        "a1d": np.broadcast_to(a1_dst.reshape(1, cfg.D1), (P, cfg.D1)).copy(),
        "a2s": np.broadcast_to(a2_src.reshape(1, cfg.D2), (P, cfg.D2)).copy(),
        "a2d": np.broadcast_to(a2_dst.reshape(1, cfg.D2), (P, cfg.D2)).copy(),
        "b1r": np.broadcast_to(b1.reshape(1, cfg.D1), (P, cfg.D1)).copy(),
        "b2r": np.broadcast_to(b2.reshape(1, cfg.D2), (P, cfg.D2)).copy(),
    }
    in_maps = []
    for c in range(cfg.NC):
        xs = x[c * cfg.SHARD:(c + 1) * cfg.SHARD]
        xTc = np.zeros((cfg.F, cfg.SHARD_PAD), dtype=BF)
        xTc[:, :cfg.SHARD] = xs.T.astype(BF)
        m = {"xT": xTc, **consts}
        for s in ("lo", "hi"):
            sW, dW, dL = ed[s]
            m[f"srcW_{s}"] = sW[c]
            m[f"dstW_{s}"] = dW[c]
            m[f"dstl_{s}"] = dL[c]
        in_maps.append(m)
    return in_maps


def assemble_output(results, cfg: Cfg):
    outs = [results[c]["out"][:cfg.SHARD] for c in range(cfg.NC)]
    return np.concatenate(outs, axis=0).astype(np.float32)


def pick_ch(edge_index: np.ndarray, cfg_kwargs: dict) -> int:
    tmp = Cfg(CH=1, TB=1, **{k: v for k, v in cfg_kwargs.items()
                             if k in ("N", "NC", "F", "H1", "HD", "D2")})
    N = tmp.N
    src = np.concatenate([np.asarray(edge_index[0]).astype(np.int64),
                          np.arange(N, dtype=np.int64)])
    dst = np.concatenate([np.asarray(edge_index[1]).astype(np.int64),
                          np.arange(N, dtype=np.int64)])
    src_remap = (src // tmp.SHARD) * tmp.SHARD_PAD + (src % tmp.SHARD)
    half = (src_remap >= tmp.VH).astype(np.int64)
    gid = ((dst // tmp.SHARD) * tmp.TILES + (dst % tmp.SHARD) // P) * 2 + half
    counts = np.bincount(gid, minlength=tmp.NC * tmp.TILES * 2)
    return int(math.ceil(counts.max() / P))


# ---------------- public entry point ----------------

_CACHE = {}


def kernel(**inputs) -> np.ndarray:
    ei = np.asarray(inputs["edge_index"]).astype(np.int64)
    ch = max(10, pick_ch(ei, dict(N=50000, NC=8, F=512)))
    cfg = Cfg(N=50000, NC=8, F=512, CH=ch, TB=7)
    key = ch
    if key not in _CACHE:
        _CACHE[key] = build_program(cfg)
    nc = _CACHE[key]
    in_maps = make_in_maps(inputs, cfg)
    from concourse import bass_utils
    res = bass_utils.run_bass_kernel_spmd(
        nc, in_maps, core_ids=list(range(cfg.NC)))
    return assemble_output(res.results, cfg)



# revision 3
# speedup vs baseline: 1.3979x; 1.3979x over previous
"""Trainium2 Bass kernel for 2-layer GAT (nn_GAT_50586124812836), v2.

Design (vs v1 one-hot/matmul):
- Host permutes nodes into (core, tile, lane) slots, balancing per-lane
  degree (2D banding).  Edge slots are laid out with partition = dst lane,
  so the per-dst attention bias is a free-dim broadcast and the weighted
  aggregation is a free-dim tensor_reduce — no one-hot builds, no PE
  aggregation matmuls, no per-edge dst-alpha gathers.
- Node table [50176, 128] bf16 (cols 0:72 layer1 h|as, cols 72:105 layer2),
  split in two blocks (24/25 tiles per core) so each block has < 32768 rows
  and SWDGE gather int16 indices address it directly.
- AllGathers are split per block and pipelined against dense/edge compute;
  both layers share the same table and the same gather index tables.
"""
import math
from dataclasses import dataclass

import numpy as np
import ml_dtypes

import concourse.bass as bass
import concourse.tile as tile
from concourse import bacc, mybir
from concourse import ap_utils
from concourse.bass import AP, MemorySpace
from concourse._compat import exact_div
from concourse.masks import make_identity
from concourse.library_config import mlp

BF16 = mybir.dt.bfloat16
I8 = mybir.dt.int8
F32 = mybir.dt.float32
I16 = mybir.dt.int16
P = 128
Alu = mybir.AluOpType
Act = mybir.ActivationFunctionType
NEG_SLOPE = 0.2
BF = ml_dtypes.bfloat16

N = 50000
NC = 8
F = 512
KC = 4            # F / 128
H1 = 8
HD = 8
D1 = 64
D2 = 32
E1 = D1 + 2 + 2 * H1   # 82 bytes: h1 i8 | sc bf16 | as1 bf16*8
E2 = D2 + 2 + 2        # 36 bytes: h2 i8 | sc bf16 | as2 bf16
L2_OFF = 128           # byte col of layer-2 row in table
TILES = 49
TB = 7
NB = 7
SHARD_PAD = TILES * P        # 6272
A_TILES = 24
A_LOC = A_TILES * P          # 3072
B_LOC = SHARD_PAD - A_LOC    # 3200
A_ROWS = NC * A_LOC          # 24576
B_ROWS = NC * B_LOC          # 25600
V = A_ROWS + B_ROWS          # 50176
PAD_IDX_A = 3071             # core0 (t23, lane127), block-1 row
PAD_IDX_B = 3199             # core0 (t48, lane127), block-2 row
STRIDED_CC = False           # BIR verifier rejects strided CC outputs
import os
NO_CC = os.environ.get("V2_NO_CC") == "1"   # timing ablation only


def dma_gather_raw(gp, out_ap: AP, in_ap: AP, idxs_ap: AP, num_idxs: int,
                   elem_size: int, elem_step: int, queue_num: int = 0,
                   single_packet: bool = False):
    assert idxs_ap.dtype == mybir.dt.int16
    assert in_ap.space == MemorySpace.DRAM
    assert idxs_ap.space == MemorySpace.SBUF
    assert out_ap.space == MemorySpace.SBUF
    assert in_ap.dtype == out_ap.dtype
    dtsz = mybir.dt.size(in_ap.dtype)
    stride_bytes_256 = exact_div(elem_step * dtsz, 256)
    assert 0 < stride_bytes_256 < 256
    assert ap_utils.ap_is_contiguous(in_ap.ap[1:])
    assert ap_utils.ap_is_contiguous(out_ap.ap[1:])
    assert ap_utils.ap_is_contiguous(idxs_ap.ap[1:])
    assert in_ap.ap[0][0] == elem_step
    assert in_ap.ap[-1][1] == elem_size
    assert out_ap.ap[-1][1] == elem_size
    assert num_idxs % 128 == 0
    assert out_ap.ap[0][1] * out_ap.ap[1][1] == num_idxs
    _in_ap = gp.lower_ap_dma(in_ap, for_custom_bir_dma=True)
    _idxs_ap = gp.lower_ap(idxs_ap)
    _out_ap = gp.lower_ap(out_ap)
    return gp.add_instruction(
        mybir.InstDMAGatherAnt(
            name=gp.bass.get_next_instruction_name(),
            ins=[*_in_ap, _idxs_ap,
                 gp.lower_val_access(gp.to_reg(num_idxs))],
            outs=[_out_ap],
            transpose=False,
            num_idxs=num_idxs,
            elem_size=elem_size,
            stride_bytes_256=stride_bytes_256,
            gen_mode=0,
            single_packet=single_packet,
            queue_num=queue_num,
            sbuf_tokens_per_rank=0,
            sbuf_free_dim_per_rank=0,
            sbuf_free_dim_pad_per_rank=0,
            sbuf_byte_offset=0,
        ))


@dataclass(frozen=True)
class V2Cfg:
    KA: tuple          # per-tile K, bucket A (len 49)
    KB: tuple          # per-tile K, bucket B

    def batch_K(self, sweep, b):
        K = self.KA if sweep == 0 else self.KB
        return [int(K[b * TB + tt]) for tt in range(TB)]


def build_program(cfg: V2Cfg):
    nc = bacc.Bacc("TRN2", target_bir_lowering=False, debug=False,
                   num_devices=NC, dynamic_dma_scratch_size=32768)
    dt = nc.dram_tensor
    xT = dt("xT", [F, SHARD_PAD], BF16, kind="ExternalInput")
    # total wrapped idx columns
    totc = 0
    seg_cols = {}
    for sweep in (0, 1):
        for b in range(NB):
            n = P * sum(cfg.batch_K(sweep, b))
            seg_cols[(sweep, b)] = (totc, n // 16)
            totc += n // 16
    srcW = dt("srcW", [16, totc], I16, kind="ExternalInput")
    w1 = dt("w1", [P, KC * D1], BF16, kind="ExternalInput")
    w2 = dt("w2", [D1, D2], BF16, kind="ExternalInput")
    a1s = dt("a1s", [P, D1], F32, kind="ExternalInput")
    a1d = dt("a1d", [P, D1], F32, kind="ExternalInput")
    a2s = dt("a2s", [P, D2], F32, kind="ExternalInput")
    a2d = dt("a2d", [P, D2], F32, kind="ExternalInput")
    b1r = dt("b1r", [P, D1], F32, kind="ExternalInput")
    b2r = dt("b2r", [P, D2], F32, kind="ExternalInput")
    padc = dt("padc", [1, 2 * H1], I8, kind="ExternalInput")

    ha1_sh = dt("ha1_sh", [SHARD_PAD, E1], I8, kind="Internal")
    ha2_sh = dt("ha2_sh", [SHARD_PAD, E2], I8, kind="Internal")
    table = dt("table", [V, 256], I8, kind="Internal", addr_space="Shared")
    tpk = {}
    if not STRIDED_CC:
        tpk[(1, 0)] = dt("tpk1a", [A_ROWS, E1], I8, kind="Internal",
                         addr_space="Shared")
        tpk[(1, 1)] = dt("tpk1b", [B_ROWS, E1], I8, kind="Internal",
                         addr_space="Shared")
        tpk[(2, 0)] = dt("tpk2a", [A_ROWS, E2], I8, kind="Internal",
                         addr_space="Shared")
        tpk[(2, 1)] = dt("tpk2b", [B_ROWS, E2], I8, kind="Internal",
                         addr_space="Shared")
    out = dt("out", [SHARD_PAD, D2], F32, kind="ExternalOutput")
    rg = [list(range(NC))]

    def allgather(layer, blk):
        src_t = ha1_sh if layer == 1 else ha2_sh
        row = E1 if layer == 1 else E2
        c0 = 0 if layer == 1 else L2_OFF
        loc = slice(0, A_LOC) if blk == 0 else slice(A_LOC, SHARD_PAD)
        rows = slice(0, A_ROWS) if blk == 0 else slice(A_ROWS, V)
        if NO_CC:
            # ablation: local copy in place of collective (wrong results)
            base = 0 if blk == 0 else A_ROWS
            nloc = A_LOC if blk == 0 else B_LOC
            nc.sync.dma_start(table[base:base + nloc, c0:c0 + row],
                              src_t[loc, :])
            return
        if STRIDED_CC:
            nc.gpsimd.collective_compute(
                "AllGather", Alu.bypass, replica_groups=rg,
                ins=[src_t[loc, :]], outs=[table[rows, c0:c0 + row]])
        else:
            tmp = tpk[(layer, blk)]
            nc.gpsimd.collective_compute(
                "AllGather", Alu.bypass, replica_groups=rg,
                ins=[src_t[loc, :]], outs=[tmp[:, :]])
            nc.sync.dma_start(table[rows, c0:c0 + row], tmp[:, :])

    with tile.TileContext(nc) as tc:
        cpool_cm = tc.tile_pool(name="consts", bufs=1)
        cpool = cpool_cm.__enter__()
        nc.gpsimd.load_library(mlp)
        w1s = cpool.tile([P, KC, D1], BF16)
        nc.sync.dma_start(w1s[:], w1[:].rearrange("p (k d) -> p k d", k=KC))
        w2s = cpool.tile([D1, D2], BF16)
        nc.sync.dma_start(w2s[:], w2[:])
        a1s_s = cpool.tile([P, D1], F32)
        nc.sync.dma_start(a1s_s[:], a1s[:])
        a1d_s = cpool.tile([P, D1], F32)
        nc.sync.dma_start(a1d_s[:], a1d[:])
        a2s_s = cpool.tile([P, D2], F32)
        nc.sync.dma_start(a2s_s[:], a2s[:])
        a2d_s = cpool.tile([P, D2], F32)
        nc.sync.dma_start(a2d_s[:], a2d[:])
        b1_s = cpool.tile([P, D1], F32)
        nc.sync.dma_start(b1_s[:], b1r[:])
        b2_s = cpool.tile([P, D2], F32)
        nc.sync.dma_start(b2_s[:], b2r[:])
        ident = cpool.tile([P, P], BF16)
        make_identity(nc, ident[:])
        iw = cpool.tile([P, totc], I16)
        for k in range(8):
            nc.sync.dma_start(iw[16 * k:16 * (k + 1), :], srcW[:, :])
        ad1_sb = cpool.tile([P, TILES, H1], F32)
        ad2_sb = cpool.tile([P, TILES, 1], F32)
        part1 = cpool.tile([P, TILES, D1 + H1], F32)
        part2 = cpool.tile([P, TILES, D2 + 1], F32)

        # ---------------- Phase A: dense layer 1 ----------------
        with tc.tile_pool(name="pA", bufs=3) as pool, \
             tc.tile_pool(name="pAps", bufs=2, space="PSUM") as pps:
            for b in range(NB):
                r0 = b * TB * P
                xt = pool.tile([P, TB, KC, P], BF16, name="xt")
                xTv = xT[:].rearrange("(k p) (t n) -> p k t n", p=P, n=P)
                for k in range(KC):
                    nc.sync.dma_start(
                        xt[:, :, k, :],
                        xTv[:, k, b * TB:(b + 1) * TB])
                h1ps = pps.tile([P, TB, D1], F32, name="h1ps")
                for tt in range(TB):
                    for k in range(KC):
                        nc.tensor.matmul(
                            out=h1ps[:, tt, :], lhsT=xt[:, tt, k, :],
                            rhs=w1s[:, k, :], start=(k == 0),
                            stop=(k == KC - 1))
                tmps = pool.tile([P, TB, D1], F32, name="tmps")
                nc.vector.tensor_tensor(
                    out=tmps[:], in0=h1ps[:],
                    in1=a1s_s[:].unsqueeze(1).broadcast_to([P, TB, D1]),
                    op=Alu.mult)
                as1 = pool.tile([P, TB, H1], F32, name="as1")
                nc.vector.tensor_reduce(
                    out=as1[:],
                    in_=tmps[:].rearrange("p t (h r) -> p t h r", h=H1),
                    axis=mybir.AxisListType.X, op=Alu.add)
                tmpd = pool.tile([P, TB, D1], F32, name="tmpd")
                nc.vector.tensor_tensor(
                    out=tmpd[:], in0=h1ps[:],
                    in1=a1d_s[:].unsqueeze(1).broadcast_to([P, TB, D1]),
                    op=Alu.mult)
                nc.vector.tensor_reduce(
                    out=ad1_sb[:, b * TB:(b + 1) * TB, :],
                    in_=tmpd[:].rearrange("p t (h r) -> p t h r", h=H1),
                    axis=mybir.AxisListType.X, op=Alu.add)
                amax = pool.tile([P, TB, 1], F32, name="amax")
                nc.vector.tensor_reduce(
                    out=amax[:], in_=h1ps[:], axis=mybir.AxisListType.X,
                    op=Alu.max, apply_absolute_value=True)
                amaxe = pool.tile([P, TB, 1], F32, name="amaxe")
                nc.vector.tensor_scalar_add(amaxe[:], amax[:], 1e-20)
                scb = pool.tile([P, TB, 1], BF16, name="scb")
                nc.vector.tensor_scalar_mul(scb[:], amaxe[:], 1.0 / 127.0)
                rcp = pool.tile([P, TB, 1], F32, name="rcp")
                nc.vector.reciprocal(rcp[:], scb[:])
                hs = pool.tile([P, TB, D1], F32, name="hs")
                nc.vector.tensor_tensor(
                    out=hs[:], in0=h1ps[:],
                    in1=rcp[:].broadcast_to([P, TB, D1]), op=Alu.mult)
                sgn = pool.tile([P, TB, D1], F32, name="sgn")
                nc.scalar.activation(sgn[:], h1ps[:], Act.Sign)
                qf = pool.tile([P, TB, D1], F32, name="qf")
                nc.vector.scalar_tensor_tensor(
                    out=qf[:], in0=sgn[:], scalar=0.25, in1=hs[:],
                    op0=Alu.mult, op1=Alu.add)
                ha = pool.tile([P, TB, E1], I8, name="ha")
                nc.vector.tensor_copy(ha[:, :, 0:D1], qf[:])
                nc.vector.tensor_copy(
                    ha[:, :, D1:D1 + 2].bitcast(BF16), scb[:])
                nc.vector.tensor_copy(
                    ha[:, :, D1 + 2:E1].bitcast(BF16), as1[:])
                nc.sync.dma_start(
                    ha1_sh[r0:r0 + TB * P, :]
                    .rearrange("(t p) c -> p t c", p=P), ha[:])
                if b == 3:    # pad row: tile 23 lane 127 -> row 3071
                    nc.sync.dma_start(
                        ha1_sh[3071:3072, D1 + 2:E1], padc[0:1, :])
                    allgather(1, 0)
                if b == 6:    # pad row: tile 48 lane 127 -> row 6271
                    nc.sync.dma_start(
                        ha1_sh[6271:6272, D1 + 2:E1], padc[0:1, :])
                    allgather(1, 1)

        def edge_sweep(layer, sweep, pool, pps):
            """sweep 0 = bucket A (block-1 srcs), 1 = bucket B."""
            if layer == 1:
                ROW, NH, D = E1, H1, D1
                c0 = 0
                ad_sb = ad1_sb
            else:
                ROW, NH, D = E2, 1, D2
                c0 = L2_OFF
                ad_sb = ad2_sb
            rows = slice(0, A_ROWS) if sweep == 0 else slice(A_ROWS, V)
            part = part1 if layer == 1 else part2
            for b in range(NB):
                Ks = cfg.batch_K(sweep, b)
                SK = sum(Ks)
                col0, ncols = seg_cols[(sweep, b)]
                G = pool.tile([P, SK, ROW], I8, name="G")
                # split into pieces of <= 48 slot-cols (<= ~385 ring descs)
                p0 = 0
                acc = 0
                for tt in range(TB + 1):
                    if tt == TB or (acc and acc + Ks[tt] > 48):
                        nidx = P * acc
                        dma_gather_raw(
                            nc.gpsimd, G[:, p0:p0 + acc, :],
                            table[rows, c0:c0 + ROW],
                            iw[:, col0 + p0 * 8:col0 + (p0 + acc) * 8],
                            nidx, ROW, 256)
                        p0 += acc
                        acc = 0
                    if tt < TB:
                        acc += Ks[tt]
                if sweep == 1:
                    pB = pool.tile([P, TB, D + NH], F32, name="pB")
                off = 0
                for tt in range(TB):
                    t = b * TB + tt
                    K = Ks[tt]
                    Gt = G[:, off:off + K, :]
                    off += K
                    asv = Gt[:, :, D + 2:ROW].bitcast(BF16)
                    scv = Gt[:, :, D:D + 2].bitcast(BF16)
                    TE = pool.tile([P, K, NH], F32, name="TE")
                    nc.vector.tensor_tensor(
                        out=TE[:], in0=asv,
                        in1=ad_sb[:, t, :].unsqueeze(1)
                        .broadcast_to([P, K, NH]), op=Alu.add)
                    LR = pool.tile([P, K, NH], F32, name="LR")
                    nc.vector.scalar_tensor_tensor(
                        out=LR[:], in0=TE[:], scalar=NEG_SLOPE, in1=TE[:],
                        op0=Alu.mult, op1=Alu.max)
                    EX = pool.tile([P, K, NH], BF16, name="EX")
                    nc.scalar.activation(EX[:], LR[:], Act.Exp)
                    EXs = pool.tile([P, K, NH], BF16, name="EXs")
                    nc.vector.tensor_tensor(
                        out=EXs[:], in0=EX[:],
                        in1=scv.broadcast_to([P, K, NH]), op=Alu.mult)
                    hb = pool.tile([P, K, D], BF16, name="hb")
                    nc.vector.tensor_copy(hb[:], Gt[:, :, 0:D])
                    R = pool.tile([P, K, D], BF16, name="R")
                    nc.vector.tensor_tensor(
                        out=R[:].rearrange("p j (h q) -> p j h q", h=NH),
                        in0=hb[:].rearrange("p j (h q) -> p j h q", h=NH),
                        in1=EXs[:].unsqueeze(3)
                        .broadcast_to([P, K, NH, D // NH]), op=Alu.mult)
                    if sweep == 0:
                        onum = part[:, t, 0:D]
                        oden = part[:, t, D:D + NH]
                    else:
                        onum = pB[:, tt, 0:D]
                        oden = pB[:, tt, D:D + NH]
                    nc.vector.tensor_reduce(
                        out=onum, in_=R[:].rearrange("p j f -> p f j"),
                        axis=mybir.AxisListType.X, op=Alu.add)
                    nc.vector.tensor_reduce(
                        out=oden, in_=EX[:].rearrange("p j h -> p h j"),
                        axis=mybir.AxisListType.X, op=Alu.add)
                if sweep == 1:
                    ts7 = slice(b * TB, (b + 1) * TB)
                    tot = pool.tile([P, TB, D + NH], F32, name="tot")
                    nc.vector.tensor_tensor(
                        out=tot[:], in0=part[:, ts7, :], in1=pB[:],
                        op=Alu.add)
                    RS = pool.tile([P, TB, NH], F32, name="RS")
                    nc.vector.reciprocal(RS[:], tot[:, :, D:D + NH])
                    zb = pool.tile([P, TB, D], F32, name="zb")
                    nc.vector.tensor_tensor(
                        out=zb[:].rearrange("p t (h q) -> p t h q", h=NH),
                        in0=tot[:, :, 0:D]
                        .rearrange("p t (h q) -> p t h q", h=NH),
                        in1=RS[:].unsqueeze(3)
                        .broadcast_to([P, TB, NH, D // NH]), op=Alu.mult)
                    if layer == 1:
                        finalize1(b, zb, pool, pps)
                    else:
                        o2 = pool.tile([P, TB, D2], F32, name="o2")
                        nc.vector.tensor_tensor(
                            out=o2[:], in0=zb[:],
                            in1=b2_s[:].unsqueeze(1)
                            .broadcast_to([P, TB, D2]), op=Alu.add)
                        r0 = b * TB * P
                        nc.sync.dma_start(
                            out[r0:r0 + TB * P, :]
                            .rearrange("(t p) c -> p t c", p=P), o2[:])

        def finalize1(b, zb, pool, pps):
            """ELU + dense layer 2 for batch b; zb = [P, TB, D1] f32."""
            zc = pool.tile([P, TB, D1], F32, name="zc")
            nc.vector.tensor_tensor(
                out=zc[:], in0=zb[:],
                in1=b1_s[:].unsqueeze(1).broadcast_to([P, TB, D1]),
                op=Alu.add)
            mn = pool.tile([P, TB, D1], F32, name="mn")
            nc.vector.tensor_scalar_min(mn[:], zc[:], 0.0)
            em = pool.tile([P, TB, D1], F32, name="em")
            nc.scalar.activation(em[:], mn[:], Act.Exp)
            rp = pool.tile([P, TB, D1], F32, name="rp")
            nc.vector.tensor_scalar_max(rp[:], zc[:], 0.0)
            zel = pool.tile([P, TB, D1], BF16, name="zel")
            nc.vector.scalar_tensor_tensor(
                out=zel[:], in0=em[:], scalar=-1.0, in1=rp[:],
                op0=Alu.add, op1=Alu.add)
            h2ps = pps.tile([P, TB, D2], F32, name="h2ps")
            for tt in range(TB):
                ztp = pps.tile([D1, P], BF16, name="ztp")
                nc.tensor.transpose(ztp[:], zel[:, tt, :], ident[:])
                zts = pool.tile([D1, P], BF16, name="zts")
                nc.scalar.copy(zts[:], ztp[:])
                nc.tensor.matmul(out=h2ps[:, tt, :], lhsT=zts[:],
                                 rhs=w2s[:], start=True, stop=True)
            t2s = pool.tile([P, TB, D2], F32, name="t2s")
            nc.vector.tensor_tensor(
                out=t2s[:], in0=h2ps[:],
                in1=a2s_s[:].unsqueeze(1).broadcast_to([P, TB, D2]),
                op=Alu.mult)
            as2 = pool.tile([P, TB, 1], F32, name="as2")
            nc.vector.tensor_reduce(
                out=as2[:], in_=t2s[:], axis=mybir.AxisListType.X,
                op=Alu.add)
            t2d = pool.tile([P, TB, D2], F32, name="t2d")
            nc.vector.tensor_tensor(
                out=t2d[:], in0=h2ps[:],
                in1=a2d_s[:].unsqueeze(1).broadcast_to([P, TB, D2]),
                op=Alu.mult)
            nc.vector.tensor_reduce(
                out=ad2_sb[:, b * TB:(b + 1) * TB, :], in_=t2d[:],
                axis=mybir.AxisListType.X, op=Alu.add)
            amax2 = pool.tile([P, TB, 1], F32, name="amax2")
            nc.vector.tensor_reduce(
                out=amax2[:], in_=h2ps[:], axis=mybir.AxisListType.X,
                op=Alu.max, apply_absolute_value=True)
            amax2e = pool.tile([P, TB, 1], F32, name="amax2e")
            nc.vector.tensor_scalar_add(amax2e[:], amax2[:], 1e-20)
            scb2 = pool.tile([P, TB, 1], BF16, name="scb2")
            nc.vector.tensor_scalar_mul(scb2[:], amax2e[:], 1.0 / 127.0)
            rcp2 = pool.tile([P, TB, 1], F32, name="rcp2")
            nc.vector.reciprocal(rcp2[:], scb2[:])
            hs2 = pool.tile([P, TB, D2], F32, name="hs2")
            nc.vector.tensor_tensor(
                out=hs2[:], in0=h2ps[:],
                in1=rcp2[:].broadcast_to([P, TB, D2]), op=Alu.mult)
            sgn2 = pool.tile([P, TB, D2], F32, name="sgn2")
            nc.scalar.activation(sgn2[:], h2ps[:], Act.Sign)
            qf2 = pool.tile([P, TB, D2], F32, name="qf2")
            nc.vector.scalar_tensor_tensor(
                out=qf2[:], in0=sgn2[:], scalar=0.25, in1=hs2[:],
                op0=Alu.mult, op1=Alu.add)
            ha2 = pool.tile([P, TB, E2], I8, name="ha2")
            nc.vector.tensor_copy(ha2[:, :, 0:D2], qf2[:])
            nc.vector.tensor_copy(
                ha2[:, :, D2:D2 + 2].bitcast(BF16), scb2[:])
            nc.vector.tensor_copy(
                ha2[:, :, D2 + 2:E2].bitcast(BF16), as2[:])
            r0 = b * TB * P
            nc.sync.dma_start(
                ha2_sh[r0:r0 + TB * P, :]
                .rearrange("(t p) c -> p t c", p=P), ha2[:])
            if b == 3:
                nc.sync.dma_start(
                    ha2_sh[3071:3072, D2 + 2:E2], padc[0:1, 0:2])
                allgather(2, 0)
            if b == 6:
                nc.sync.dma_start(
                    ha2_sh[6271:6272, D2 + 2:E2], padc[0:1, 0:2])
                allgather(2, 1)

        with tc.tile_pool(name="e1a", bufs=2) as pool, \
             tc.tile_pool(name="e1aps", bufs=2, space="PSUM") as pps:
            edge_sweep(1, 0, pool, pps)
        with tc.tile_pool(name="e1b", bufs=2) as pool, \
             tc.tile_pool(name="e1bps", bufs=4, space="PSUM") as pps:
            edge_sweep(1, 1, pool, pps)
        with tc.tile_pool(name="e2a", bufs=2) as pool, \
             tc.tile_pool(name="e2aps", bufs=2, space="PSUM") as pps:
            edge_sweep(2, 0, pool, pps)
        with tc.tile_pool(name="e2b", bufs=2) as pool, \
             tc.tile_pool(name="e2bps", bufs=2, space="PSUM") as pps:
            edge_sweep(2, 1, pool, pps)
        cpool_cm.__exit__(None, None, None)

    nc.compile()
    return nc


# ---------------- host-side preprocessing ----------------

def build_assignment(edge_index):
    src0 = np.asarray(edge_index[0]).astype(np.int64)
    dst0 = np.asarray(edge_index[1]).astype(np.int64)
    loops = np.arange(N, dtype=np.int64)
    src = np.concatenate([src0, loops])
    dst = np.concatenate([dst0, loops])

    deg = np.bincount(dst, minlength=N)
    order = np.argsort(-deg, kind="stable")

    q = np.arange(TILES * 1024)
    t_all = q // 1024
    qq = q % 1024
    c_all = qq % NC
    l_all = qq // NC
    keep = ~(((t_all == 23) | (t_all == 48)) & (l_all == 127))
    slot_t = t_all[keep][:N]
    slot_c = c_all[keep][:N]
    slot_l = l_all[keep][:N]

    n_a_slots = int((slot_t < A_TILES).sum())
    a_nodes = np.zeros(N, bool)
    a_nodes[order[:n_a_slots]] = True
    deg_a = np.bincount(dst[a_nodes[src]], minlength=N)

    counts = np.full(TILES, 1024, np.int64)
    counts[23] = counts[48] = 1016
    cum = np.concatenate([[0], np.cumsum(counts)])
    pick = order.copy()
    for band0 in range(0, TILES, 8):
        s0 = int(cum[band0])
        s1 = min(int(cum[min(band0 + 8, TILES)]), N)
        if s0 >= N:
            break
        seg = pick[s0:s1]
        pick[s0:s1] = seg[np.argsort(-deg_a[seg], kind="stable")]

    core_of = np.empty(N, np.int64)
    tile_of = np.empty(N, np.int64)
    lane_of = np.empty(N, np.int64)
    core_of[pick] = slot_c
    tile_of[pick] = slot_t
    lane_of[pick] = slot_l
    return src, dst, core_of, tile_of, lane_of


def preprocess(edge_index):
    src, dst, core_of, tile_of, lane_of = build_assignment(edge_index)
    local_of = tile_of * P + lane_of
    grow = np.where(local_of < A_LOC, core_of * A_LOC + local_of,
                    A_ROWS + core_of * B_LOC + (local_of - A_LOC))
    sg = grow[src]
    bkt = (sg >= A_ROWS).astype(np.int64)
    idxval = (sg - bkt * A_ROWS).astype(np.int64)
    dc = core_of[dst]
    dt_ = tile_of[dst]
    dl = lane_of[dst]

    key = ((dc * TILES + dt_) * 2 + bkt) * P + dl
    ordr = np.argsort(key, kind="stable")
    ks = key[ordr]
    iv = idxval[ordr]
    nkeys = NC * TILES * 2 * P
    cnt = np.bincount(key, minlength=nkeys)
    starts = np.zeros(nkeys + 1, np.int64)
    np.cumsum(cnt, out=starts[1:])
    j = np.arange(len(ks)) - starts[ks]

    cnt4 = cnt.reshape(NC, TILES, 2, P)
    KA = cnt4[:, :, 0, :].max(axis=(0, 2)).astype(np.int64)
    KB = cnt4[:, :, 1, :].max(axis=(0, 2)).astype(np.int64)

    # flat slot streams per (core, sweep): [128 * sum(K)] with per-batch
    # contiguous segments; position = seg_base + (off_t + j)*128 + lane
    def stream_layout(K):
        offt = np.zeros(TILES, np.int64)     # col offset within batch
        segb = np.zeros(NB + 1, np.int64)    # slot base of batch segment
        for b in range(NB):
            o = 0
            for tt in range(TB):
                offt[b * TB + tt] = o
                o += int(K[b * TB + tt])
            segb[b + 1] = segb[b] + P * o
        return offt, segb

    offA, segA = stream_layout(KA)
    offB, segB = stream_layout(KB)
    streams = np.empty((NC, 2), object)
    for c in range(NC):
        streams[c, 0] = np.full(int(segA[NB]), PAD_IDX_A, np.int16)
        streams[c, 1] = np.full(int(segB[NB]), PAD_IDX_B, np.int16)
    kc = ks // (TILES * 2 * P)
    kt = (ks // (2 * P)) % TILES
    kb = (ks // P) % 2
    kl = ks % P
    bb = kt // TB
    offt_of = np.where(kb == 0, offA[kt], offB[kt])
    segb_of = np.where(kb == 0, segA[bb], segB[bb])
    pos = segb_of + (offt_of + j) * P + kl
    for c in range(NC):
        for s in (0, 1):
            m = (kc == c) & (kb == s)
            streams[c, s][pos[m]] = iv[m].astype(np.int16)

    # wrap each (sweep, batch) segment into [16, n/16] and concat cols
    srcw = []
    for c in range(NC):
        parts = []
        for s in (0, 1):
            seg = segA if s == 0 else segB
            for b in range(NB):
                fl = streams[c, s][seg[b]:seg[b + 1]]
                parts.append(fl.reshape(-1, 16).T)
        srcw.append(np.ascontiguousarray(np.concatenate(parts, axis=1)))
    cfg = V2Cfg(KA=tuple(int(k) for k in KA), KB=tuple(int(k) for k in KB))
    return cfg, srcw, core_of, local_of


def make_in_maps(inputs, cfg, srcw, core_of, local_of):
    x = np.asarray(inputs["x"], dtype=np.float32)
    W1 = np.asarray(inputs["W1"], dtype=np.float32)
    a1_src = np.asarray(inputs["a1_src"], dtype=np.float32).reshape(1, D1)
    a1_dst = np.asarray(inputs["a1_dst"], dtype=np.float32).reshape(1, D1)
    b1 = np.asarray(inputs["b1"], dtype=np.float32).reshape(1, D1)
    W2 = np.asarray(inputs["W2"], dtype=np.float32)
    a2_src = np.asarray(inputs["a2_src"], dtype=np.float32).reshape(1, D2)
    a2_dst = np.asarray(inputs["a2_dst"], dtype=np.float32).reshape(1, D2)
    b2 = np.asarray(inputs["b2"], dtype=np.float32).reshape(1, D2)

    w1_dev = np.ascontiguousarray(
        W1.reshape(KC, P, D1).transpose(1, 0, 2).reshape(P, KC * D1)
    ).astype(BF)
    consts = {
        "w1": w1_dev, "w2": W2.astype(BF),
        "a1s": np.broadcast_to(a1_src, (P, D1)).copy(),
        "a1d": np.broadcast_to(a1_dst, (P, D1)).copy(),
        "a2s": np.broadcast_to(a2_src, (P, D2)).copy(),
        "a2d": np.broadcast_to(a2_dst, (P, D2)).copy(),
        "b1r": np.broadcast_to(b1, (P, D1)).copy(),
        "b2r": np.broadcast_to(b2, (P, D2)).copy(),
        "padc": np.full(H1, -30.0, dtype=BF).view(np.int8).reshape(1, 2 * H1),
    }
    xbf = x.astype(BF)
    in_maps = []
    for c in range(NC):
        nodes = np.where(core_of == c)[0]
        xTc = np.zeros((F, SHARD_PAD), dtype=BF)
        xTc[:, local_of[nodes]] = xbf[nodes].T
        in_maps.append({"xT": xTc, "srcW": srcw[c], **consts})
    return in_maps


def assemble_output(results, core_of, local_of):
    outg = np.zeros((N, D2), np.float32)
    for c in range(NC):
        nodes = np.where(core_of == c)[0]
        outg[nodes] = results[c]["out"][local_of[nodes]]
    return outg


# ---------------- public entry point ----------------

_CACHE = {}


def kernel(**inputs) -> np.ndarray:
    ei = np.asarray(inputs["edge_index"]).astype(np.int64)
    cfg, srcw, core_of, local_of = preprocess(ei)
    if cfg not in _CACHE:
        _CACHE[cfg] = build_program(cfg)
    nc = _CACHE[cfg]
    in_maps = make_in_maps(inputs, cfg, srcw, core_of, local_of)
    from concourse import bass_utils
    res = bass_utils.run_bass_kernel_spmd(
        nc, in_maps, core_ids=list(range(NC)))
    return assemble_output(res.results, core_of, local_of)


# ---------------- bench harness hooks ----------------

def bench_build(inputs):
    ei = np.asarray(inputs["edge_index"]).astype(np.int64)
    cfg, srcw, core_of, local_of = preprocess(ei)
    nc = build_program(cfg)
    in_maps = make_in_maps(inputs, cfg, srcw, core_of, local_of)
    return nc, in_maps, (core_of, local_of)


def bench_assemble(outs, out_names, out_avals, n_cores, ctx):
    core_of, local_of = ctx
    i = out_names.index("out")
    arr = np.asarray(outs[i]).reshape(n_cores, *out_avals[i].shape)
    results = [{"out": arr[c]} for c in range(n_cores)]
    return assemble_output(results, core_of, local_of)


# revision 4
# speedup vs baseline: 1.4520x; 1.0387x over previous
"""Trainium2 Bass kernel for 2-layer GAT (nn_GAT_50586124812836), v2.

Design (vs v1 one-hot/matmul):
- Host permutes nodes into (core, tile, lane) slots, balancing per-lane
  degree (2D banding).  Edge slots are laid out with partition = dst lane,
  so the per-dst attention bias is a free-dim broadcast and the weighted
  aggregation is a free-dim tensor_reduce — no one-hot builds, no PE
  aggregation matmuls, no per-edge dst-alpha gathers.
- Node table [50176, 128] bf16 (cols 0:72 layer1 h|as, cols 72:105 layer2),
  split in two blocks (24/25 tiles per core) so each block has < 32768 rows
  and SWDGE gather int16 indices address it directly.
- AllGathers are split per block and pipelined against dense/edge compute;
  both layers share the same table and the same gather index tables.
"""
import math
from dataclasses import dataclass

import numpy as np
import ml_dtypes

import concourse.bass as bass
import concourse.tile as tile
from concourse import bacc, mybir
from concourse import ap_utils
from concourse.bass import AP, MemorySpace
from concourse._compat import exact_div
from concourse.masks import make_identity
from concourse.library_config import mlp

BF16 = mybir.dt.bfloat16
I8 = mybir.dt.int8
F32 = mybir.dt.float32
I16 = mybir.dt.int16
P = 128
Alu = mybir.AluOpType
Act = mybir.ActivationFunctionType
NEG_SLOPE = 0.2
BF = ml_dtypes.bfloat16

N = 50000
NC = 8
F = 512
KC = 4            # F / 128
H1 = 8
HD = 8
D1 = 64
D2 = 32
E1 = D1 + 2 + 2 * H1   # 82 bytes: h1 i8 | sc bf16 | as1 bf16*8
E2 = D2 + 2 + 2        # 36 bytes: h2 i8 | sc bf16 | as2 bf16
L2_OFF = 128           # byte col of layer-2 row in table
TILES = 49
TB = 7
NB = 7
SHARD_PAD = TILES * P        # 6272
A_TILES = 24
A_LOC = A_TILES * P          # 3072
B_LOC = SHARD_PAD - A_LOC    # 3200
A_ROWS = NC * A_LOC          # 24576
B_ROWS = NC * B_LOC          # 25600
V = A_ROWS + B_ROWS          # 50176
PAD_IDX_A = 3071             # core0 (t23, lane127), block-1 row
PAD_IDX_B = 3199             # core0 (t48, lane127), block-2 row
STRIDED_CC = False           # BIR verifier rejects strided CC outputs
NSWQ = 4                     # spread gathers across SWDGE queues
import os
NO_CC = os.environ.get("V2_NO_CC") == "1"   # timing ablation only


def dma_gather_raw(gp, out_ap: AP, in_ap: AP, idxs_ap: AP, num_idxs: int,
                   elem_size: int, elem_step: int, queue_num: int = 0,
                   single_packet: bool = False):
    assert idxs_ap.dtype == mybir.dt.int16
    assert in_ap.space == MemorySpace.DRAM
    assert idxs_ap.space == MemorySpace.SBUF
    assert out_ap.space == MemorySpace.SBUF
    assert in_ap.dtype == out_ap.dtype
    dtsz = mybir.dt.size(in_ap.dtype)
    stride_bytes_256 = exact_div(elem_step * dtsz, 256)
    assert 0 < stride_bytes_256 < 256
    assert ap_utils.ap_is_contiguous(in_ap.ap[1:])
    assert ap_utils.ap_is_contiguous(out_ap.ap[1:])
    assert ap_utils.ap_is_contiguous(idxs_ap.ap[1:])
    assert in_ap.ap[0][0] == elem_step
    assert in_ap.ap[-1][1] == elem_size
    assert out_ap.ap[-1][1] == elem_size
    assert num_idxs % 128 == 0
    assert out_ap.ap[0][1] * out_ap.ap[1][1] == num_idxs
    _in_ap = gp.lower_ap_dma(in_ap, for_custom_bir_dma=True)
    _idxs_ap = gp.lower_ap(idxs_ap)
    _out_ap = gp.lower_ap(out_ap)
    return gp.add_instruction(
        mybir.InstDMAGatherAnt(
            name=gp.bass.get_next_instruction_name(),
            ins=[*_in_ap, _idxs_ap,
                 gp.lower_val_access(gp.to_reg(num_idxs))],
            outs=[_out_ap],
            transpose=False,
            num_idxs=num_idxs,
            elem_size=elem_size,
            stride_bytes_256=stride_bytes_256,
            gen_mode=0,
            single_packet=single_packet,
            queue_num=queue_num,
            sbuf_tokens_per_rank=0,
            sbuf_free_dim_per_rank=0,
            sbuf_free_dim_pad_per_rank=0,
            sbuf_byte_offset=0,
        ))


@dataclass(frozen=True)
class V2Cfg:
    KA: tuple          # per-tile K, bucket A (len 49)
    KB: tuple          # per-tile K, bucket B

    def batch_K(self, sweep, b):
        K = self.KA if sweep == 0 else self.KB
        return [int(K[b * TB + tt]) for tt in range(TB)]


def build_program(cfg: V2Cfg):
    nc = bacc.Bacc("TRN2", target_bir_lowering=False, debug=False,
                   num_devices=NC, dynamic_dma_scratch_size=32768,
                   num_swdge_queues=NSWQ)
    dt = nc.dram_tensor
    xT = dt("xT", [F, SHARD_PAD], BF16, kind="ExternalInput")
    # total wrapped idx columns
    totc = 0
    seg_cols = {}
    for sweep in (0, 1):
        for b in range(NB):
            n = P * sum(cfg.batch_K(sweep, b))
            seg_cols[(sweep, b)] = (totc, n // 16)
            totc += n // 16
    srcW = dt("srcW", [16, totc], I16, kind="ExternalInput")
    w1 = dt("w1", [P, KC * D1], BF16, kind="ExternalInput")
    w2 = dt("w2", [D1, D2], BF16, kind="ExternalInput")
    a1s = dt("a1s", [P, D1], F32, kind="ExternalInput")
    a1d = dt("a1d", [P, D1], F32, kind="ExternalInput")
    a2s = dt("a2s", [P, D2], F32, kind="ExternalInput")
    a2d = dt("a2d", [P, D2], F32, kind="ExternalInput")
    b1r = dt("b1r", [P, D1], F32, kind="ExternalInput")
    b2r = dt("b2r", [P, D2], F32, kind="ExternalInput")
    padc = dt("padc", [1, 2 * H1], I8, kind="ExternalInput")

    ha1_sh = dt("ha1_sh", [SHARD_PAD, E1], I8, kind="Internal")
    ha2_sh = dt("ha2_sh", [SHARD_PAD, E2], I8, kind="Internal")
    table = dt("table", [V, 256], I8, kind="Internal", addr_space="Shared")
    tpk = {}
    if not STRIDED_CC:
        tpk[(1, 0)] = dt("tpk1a", [A_ROWS, E1], I8, kind="Internal",
                         addr_space="Shared")
        tpk[(1, 1)] = dt("tpk1b", [B_ROWS, E1], I8, kind="Internal",
                         addr_space="Shared")
        tpk[(2, 0)] = dt("tpk2a", [A_ROWS, E2], I8, kind="Internal",
                         addr_space="Shared")
        tpk[(2, 1)] = dt("tpk2b", [B_ROWS, E2], I8, kind="Internal",
                         addr_space="Shared")
    out = dt("out", [SHARD_PAD, D2], F32, kind="ExternalOutput")
    rg = [list(range(NC))]

    def allgather(layer, blk):
        src_t = ha1_sh if layer == 1 else ha2_sh
        row = E1 if layer == 1 else E2
        c0 = 0 if layer == 1 else L2_OFF
        loc = slice(0, A_LOC) if blk == 0 else slice(A_LOC, SHARD_PAD)
        rows = slice(0, A_ROWS) if blk == 0 else slice(A_ROWS, V)
        if NO_CC:
            # ablation: local copy in place of collective (wrong results)
            base = 0 if blk == 0 else A_ROWS
            nloc = A_LOC if blk == 0 else B_LOC
            nc.sync.dma_start(table[base:base + nloc, c0:c0 + row],
                              src_t[loc, :])
            return
        if STRIDED_CC:
            nc.gpsimd.collective_compute(
                "AllGather", Alu.bypass, replica_groups=rg,
                ins=[src_t[loc, :]], outs=[table[rows, c0:c0 + row]])
        else:
            tmp = tpk[(layer, blk)]
            nc.gpsimd.collective_compute(
                "AllGather", Alu.bypass, replica_groups=rg,
                ins=[src_t[loc, :]], outs=[tmp[:, :]])
            nc.sync.dma_start(table[rows, c0:c0 + row], tmp[:, :])

    with tile.TileContext(nc) as tc:
        cpool_cm = tc.tile_pool(name="consts", bufs=1)
        cpool = cpool_cm.__enter__()
        nc.gpsimd.load_library(mlp)
        w1s = cpool.tile([P, KC, D1], BF16)
        nc.sync.dma_start(w1s[:], w1[:].rearrange("p (k d) -> p k d", k=KC))
        w2s = cpool.tile([D1, D2], BF16)
        nc.sync.dma_start(w2s[:], w2[:])
        a1s_s = cpool.tile([P, D1], F32)
        nc.sync.dma_start(a1s_s[:], a1s[:])
        a1d_s = cpool.tile([P, D1], F32)
        nc.sync.dma_start(a1d_s[:], a1d[:])
        a2s_s = cpool.tile([P, D2], F32)
        nc.sync.dma_start(a2s_s[:], a2s[:])
        a2d_s = cpool.tile([P, D2], F32)
        nc.sync.dma_start(a2d_s[:], a2d[:])
        b1_s = cpool.tile([P, D1], F32)
        nc.sync.dma_start(b1_s[:], b1r[:])
        b2_s = cpool.tile([P, D2], F32)
        nc.sync.dma_start(b2_s[:], b2r[:])
        ident = cpool.tile([P, P], BF16)
        make_identity(nc, ident[:])
        iw = cpool.tile([P, totc], I16)
        for k in range(8):
            nc.sync.dma_start(iw[16 * k:16 * (k + 1), :], srcW[:, :])
        ad1_sb = cpool.tile([P, TILES, H1], F32)
        ad2_sb = cpool.tile([P, TILES, 1], F32)
        part1 = cpool.tile([P, TILES, D1 + H1], F32)
        part2 = cpool.tile([P, TILES, D2 + 1], F32)

        # ---------------- Phase A: dense layer 1 ----------------
        with tc.tile_pool(name="pA", bufs=3) as pool, \
             tc.tile_pool(name="pAps", bufs=2, space="PSUM") as pps:
            for b in range(NB):
                r0 = b * TB * P
                xt = pool.tile([P, TB, KC, P], BF16, name="xt")
                xTv = xT[:].rearrange("(k p) (t n) -> p k t n", p=P, n=P)
                for k in range(KC):
                    nc.sync.dma_start(
                        xt[:, :, k, :],
                        xTv[:, k, b * TB:(b + 1) * TB])
                h1ps = pps.tile([P, TB, D1], F32, name="h1ps")
                for tt in range(TB):
                    for k in range(KC):
                        nc.tensor.matmul(
                            out=h1ps[:, tt, :], lhsT=xt[:, tt, k, :],
                            rhs=w1s[:, k, :], start=(k == 0),
                            stop=(k == KC - 1))
                tmps = pool.tile([P, TB, D1], F32, name="tmps")
                nc.vector.tensor_tensor(
                    out=tmps[:], in0=h1ps[:],
                    in1=a1s_s[:].unsqueeze(1).broadcast_to([P, TB, D1]),
                    op=Alu.mult)
                as1 = pool.tile([P, TB, H1], F32, name="as1")
                nc.vector.tensor_reduce(
                    out=as1[:],
                    in_=tmps[:].rearrange("p t (h r) -> p t h r", h=H1),
                    axis=mybir.AxisListType.X, op=Alu.add)
                tmpd = pool.tile([P, TB, D1], F32, name="tmpd")
                nc.vector.tensor_tensor(
                    out=tmpd[:], in0=h1ps[:],
                    in1=a1d_s[:].unsqueeze(1).broadcast_to([P, TB, D1]),
                    op=Alu.mult)
                nc.vector.tensor_reduce(
                    out=ad1_sb[:, b * TB:(b + 1) * TB, :],
                    in_=tmpd[:].rearrange("p t (h r) -> p t h r", h=H1),
                    axis=mybir.AxisListType.X, op=Alu.add)
                amax = pool.tile([P, TB, 1], F32, name="amax")
                nc.vector.tensor_reduce(
                    out=amax[:], in_=h1ps[:], axis=mybir.AxisListType.X,
                    op=Alu.max, apply_absolute_value=True)
                amaxe = pool.tile([P, TB, 1], F32, name="amaxe")
                nc.vector.tensor_scalar_add(amaxe[:], amax[:], 1e-20)
                scb = pool.tile([P, TB, 1], BF16, name="scb")
                nc.vector.tensor_scalar_mul(scb[:], amaxe[:], 1.0 / 127.0)
                rcp = pool.tile([P, TB, 1], F32, name="rcp")
                nc.vector.reciprocal(rcp[:], scb[:])
                hs = pool.tile([P, TB, D1], F32, name="hs")
                nc.vector.tensor_tensor(
                    out=hs[:], in0=h1ps[:],
                    in1=rcp[:].broadcast_to([P, TB, D1]), op=Alu.mult)
                sgn = pool.tile([P, TB, D1], F32, name="sgn")
                nc.scalar.activation(sgn[:], h1ps[:], Act.Sign)
                qf = pool.tile([P, TB, D1], F32, name="qf")
                nc.vector.scalar_tensor_tensor(
                    out=qf[:], in0=sgn[:], scalar=0.25, in1=hs[:],
                    op0=Alu.mult, op1=Alu.add)
                ha = pool.tile([P, TB, E1], I8, name="ha")
                nc.vector.tensor_copy(ha[:, :, 0:D1], qf[:])
                nc.vector.tensor_copy(
                    ha[:, :, D1:D1 + 2].bitcast(BF16), scb[:])
                nc.vector.tensor_copy(
                    ha[:, :, D1 + 2:E1].bitcast(BF16), as1[:])
                nc.sync.dma_start(
                    ha1_sh[r0:r0 + TB * P, :]
                    .rearrange("(t p) c -> p t c", p=P), ha[:])
                if b == 3:    # pad row: tile 23 lane 127 -> row 3071
                    nc.sync.dma_start(
                        ha1_sh[3071:3072, D1 + 2:E1], padc[0:1, :])
                    allgather(1, 0)
                if b == 6:    # pad row: tile 48 lane 127 -> row 6271
                    nc.sync.dma_start(
                        ha1_sh[6271:6272, D1 + 2:E1], padc[0:1, :])
                    allgather(1, 1)

        qctr = [0]

        def edge_sweep(layer, sweep, pool, pps):
            """sweep 0 = bucket A (block-1 srcs), 1 = bucket B."""
            if layer == 1:
                ROW, NH, D = E1, H1, D1
                c0 = 0
                ad_sb = ad1_sb
            else:
                ROW, NH, D = E2, 1, D2
                c0 = L2_OFF
                ad_sb = ad2_sb
            rows = slice(0, A_ROWS) if sweep == 0 else slice(A_ROWS, V)
            part = part1 if layer == 1 else part2
            for b in range(NB):
                Ks = cfg.batch_K(sweep, b)
                SK = sum(Ks)
                col0, ncols = seg_cols[(sweep, b)]
                G = pool.tile([P, SK, ROW], I8, name="G")
                # split into pieces of <= 48 slot-cols (<= ~385 ring descs)
                p0 = 0
                acc = 0
                for tt in range(TB + 1):
                    if tt == TB or (acc and acc + Ks[tt] > 48):
                        nidx = P * acc
                        dma_gather_raw(
                            nc.gpsimd, G[:, p0:p0 + acc, :],
                            table[rows, c0:c0 + ROW],
                            iw[:, col0 + p0 * 8:col0 + (p0 + acc) * 8],
                            nidx, ROW, 256,
                            queue_num=qctr[0] % NSWQ)
                        qctr[0] += 1
                        p0 += acc
                        acc = 0
                    if tt < TB:
                        acc += Ks[tt]
                if sweep == 1:
                    pB = pool.tile([P, TB, D + NH], F32, name="pB")
                off = 0
                for tt in range(TB):
                    t = b * TB + tt
                    K = Ks[tt]
                    Gt = G[:, off:off + K, :]
                    off += K
                    asv = Gt[:, :, D + 2:ROW].bitcast(BF16)
                    scv = Gt[:, :, D:D + 2].bitcast(BF16)
                    TE = pool.tile([P, K, NH], F32, name="TE")
                    nc.vector.tensor_tensor(
                        out=TE[:], in0=asv,
                        in1=ad_sb[:, t, :].unsqueeze(1)
                        .broadcast_to([P, K, NH]), op=Alu.add)
                    LR = pool.tile([P, K, NH], F32, name="LR")
                    nc.vector.scalar_tensor_tensor(
                        out=LR[:], in0=TE[:], scalar=NEG_SLOPE, in1=TE[:],
                        op0=Alu.mult, op1=Alu.max)
                    EX = pool.tile([P, K, NH], BF16, name="EX")
                    nc.scalar.activation(EX[:], LR[:], Act.Exp)
                    EXs = pool.tile([P, K, NH], BF16, name="EXs")
                    nc.vector.tensor_tensor(
                        out=EXs[:], in0=EX[:],
                        in1=scv.broadcast_to([P, K, NH]), op=Alu.mult)
                    hb = pool.tile([P, K, D], BF16, name="hb")
                    nc.vector.tensor_copy(hb[:], Gt[:, :, 0:D])
                    R = pool.tile([P, K, D], BF16, name="R")
                    nc.vector.tensor_tensor(
                        out=R[:].rearrange("p j (h q) -> p j h q", h=NH),
                        in0=hb[:].rearrange("p j (h q) -> p j h q", h=NH),
                        in1=EXs[:].unsqueeze(3)
                        .broadcast_to([P, K, NH, D // NH]), op=Alu.mult)
                    if sweep == 0:
                        onum = part[:, t, 0:D]
                        oden = part[:, t, D:D + NH]
                    else:
                        onum = pB[:, tt, 0:D]
                        oden = pB[:, tt, D:D + NH]
                    nc.vector.tensor_reduce(
                        out=onum, in_=R[:].rearrange("p j f -> p f j"),
                        axis=mybir.AxisListType.X, op=Alu.add)
                    nc.vector.tensor_reduce(
                        out=oden, in_=EX[:].rearrange("p j h -> p h j"),
                        axis=mybir.AxisListType.X, op=Alu.add)
                if sweep == 1:
                    ts7 = slice(b * TB, (b + 1) * TB)
                    tot = pool.tile([P, TB, D + NH], F32, name="tot")
                    nc.vector.tensor_tensor(
                        out=tot[:], in0=part[:, ts7, :], in1=pB[:],
                        op=Alu.add)
                    RS = pool.tile([P, TB, NH], F32, name="RS")
                    nc.vector.reciprocal(RS[:], tot[:, :, D:D + NH])
                    zb = pool.tile([P, TB, D], F32, name="zb")
                    nc.vector.tensor_tensor(
                        out=zb[:].rearrange("p t (h q) -> p t h q", h=NH),
                        in0=tot[:, :, 0:D]
                        .rearrange("p t (h q) -> p t h q", h=NH),
                        in1=RS[:].unsqueeze(3)
                        .broadcast_to([P, TB, NH, D // NH]), op=Alu.mult)
                    if layer == 1:
                        finalize1(b, zb, pool, pps)
                    else:
                        o2 = pool.tile([P, TB, D2], F32, name="o2")
                        nc.vector.tensor_tensor(
                            out=o2[:], in0=zb[:],
                            in1=b2_s[:].unsqueeze(1)
                            .broadcast_to([P, TB, D2]), op=Alu.add)
                        r0 = b * TB * P
                        nc.sync.dma_start(
                            out[r0:r0 + TB * P, :]
                            .rearrange("(t p) c -> p t c", p=P), o2[:])

        def finalize1(b, zb, pool, pps):
            """ELU + dense layer 2 for batch b; zb = [P, TB, D1] f32."""
            zc = pool.tile([P, TB, D1], F32, name="zc")
            nc.vector.tensor_tensor(
                out=zc[:], in0=zb[:],
                in1=b1_s[:].unsqueeze(1).broadcast_to([P, TB, D1]),
                op=Alu.add)
            mn = pool.tile([P, TB, D1], F32, name="mn")
            nc.vector.tensor_scalar_min(mn[:], zc[:], 0.0)
            em = pool.tile([P, TB, D1], F32, name="em")
            nc.scalar.activation(em[:], mn[:], Act.Exp)
            rp = pool.tile([P, TB, D1], F32, name="rp")
            nc.vector.tensor_scalar_max(rp[:], zc[:], 0.0)
            zel = pool.tile([P, TB, D1], BF16, name="zel")
            nc.vector.scalar_tensor_tensor(
                out=zel[:], in0=em[:], scalar=-1.0, in1=rp[:],
                op0=Alu.add, op1=Alu.add)
            h2ps = pps.tile([P, TB, D2], F32, name="h2ps")
            for tt in range(TB):
                ztp = pps.tile([D1, P], BF16, name="ztp")
                nc.tensor.transpose(ztp[:], zel[:, tt, :], ident[:])
                zts = pool.tile([D1, P], BF16, name="zts")
                nc.scalar.copy(zts[:], ztp[:])
                nc.tensor.matmul(out=h2ps[:, tt, :], lhsT=zts[:],
                                 rhs=w2s[:], start=True, stop=True)
            t2s = pool.tile([P, TB, D2], F32, name="t2s")
            nc.vector.tensor_tensor(
                out=t2s[:], in0=h2ps[:],
                in1=a2s_s[:].unsqueeze(1).broadcast_to([P, TB, D2]),
                op=Alu.mult)
            as2 = pool.tile([P, TB, 1], F32, name="as2")
            nc.vector.tensor_reduce(
                out=as2[:], in_=t2s[:], axis=mybir.AxisListType.X,
                op=Alu.add)
            t2d = pool.tile([P, TB, D2], F32, name="t2d")
            nc.vector.tensor_tensor(
                out=t2d[:], in0=h2ps[:],
                in1=a2d_s[:].unsqueeze(1).broadcast_to([P, TB, D2]),
                op=Alu.mult)
            nc.vector.tensor_reduce(
                out=ad2_sb[:, b * TB:(b + 1) * TB, :], in_=t2d[:],
                axis=mybir.AxisListType.X, op=Alu.add)
            amax2 = pool.tile([P, TB, 1], F32, name="amax2")
            nc.vector.tensor_reduce(
                out=amax2[:], in_=h2ps[:], axis=mybir.AxisListType.X,
                op=Alu.max, apply_absolute_value=True)
            amax2e = pool.tile([P, TB, 1], F32, name="amax2e")
            nc.vector.tensor_scalar_add(amax2e[:], amax2[:], 1e-20)
            scb2 = pool.tile([P, TB, 1], BF16, name="scb2")
            nc.vector.tensor_scalar_mul(scb2[:], amax2e[:], 1.0 / 127.0)
            rcp2 = pool.tile([P, TB, 1], F32, name="rcp2")
            nc.vector.reciprocal(rcp2[:], scb2[:])
            hs2 = pool.tile([P, TB, D2], F32, name="hs2")
            nc.vector.tensor_tensor(
                out=hs2[:], in0=h2ps[:],
                in1=rcp2[:].broadcast_to([P, TB, D2]), op=Alu.mult)
            sgn2 = pool.tile([P, TB, D2], F32, name="sgn2")
            nc.scalar.activation(sgn2[:], h2ps[:], Act.Sign)
            qf2 = pool.tile([P, TB, D2], F32, name="qf2")
            nc.vector.scalar_tensor_tensor(
                out=qf2[:], in0=sgn2[:], scalar=0.25, in1=hs2[:],
                op0=Alu.mult, op1=Alu.add)
            ha2 = pool.tile([P, TB, E2], I8, name="ha2")
            nc.vector.tensor_copy(ha2[:, :, 0:D2], qf2[:])
            nc.vector.tensor_copy(
                ha2[:, :, D2:D2 + 2].bitcast(BF16), scb2[:])
            nc.vector.tensor_copy(
                ha2[:, :, D2 + 2:E2].bitcast(BF16), as2[:])
            r0 = b * TB * P
            nc.sync.dma_start(
                ha2_sh[r0:r0 + TB * P, :]
                .rearrange("(t p) c -> p t c", p=P), ha2[:])
            if b == 3:
                nc.sync.dma_start(
                    ha2_sh[3071:3072, D2 + 2:E2], padc[0:1, 0:2])
                allgather(2, 0)
            if b == 6:
                nc.sync.dma_start(
                    ha2_sh[6271:6272, D2 + 2:E2], padc[0:1, 0:2])
                allgather(2, 1)

        with tc.tile_pool(name="e1a", bufs=2) as pool, \
             tc.tile_pool(name="e1aps", bufs=2, space="PSUM") as pps:
            edge_sweep(1, 0, pool, pps)
        with tc.tile_pool(name="e1b", bufs=2) as pool, \
             tc.tile_pool(name="e1bps", bufs=4, space="PSUM") as pps:
            edge_sweep(1, 1, pool, pps)
        with tc.tile_pool(name="e2a", bufs=2) as pool, \
             tc.tile_pool(name="e2aps", bufs=2, space="PSUM") as pps:
            edge_sweep(2, 0, pool, pps)
        with tc.tile_pool(name="e2b", bufs=2) as pool, \
             tc.tile_pool(name="e2bps", bufs=2, space="PSUM") as pps:
            edge_sweep(2, 1, pool, pps)
        cpool_cm.__exit__(None, None, None)

    nc.compile()
    return nc


# ---------------- host-side preprocessing ----------------

def build_assignment(edge_index):
    src0 = np.asarray(edge_index[0]).astype(np.int64)
    dst0 = np.asarray(edge_index[1]).astype(np.int64)
    loops = np.arange(N, dtype=np.int64)
    src = np.concatenate([src0, loops])
    dst = np.concatenate([dst0, loops])

    deg = np.bincount(dst, minlength=N)
    order = np.argsort(-deg, kind="stable")

    q = np.arange(TILES * 1024)
    t_all = q // 1024
    qq = q % 1024
    c_all = qq % NC
    l_all = qq // NC
    keep = ~(((t_all == 23) | (t_all == 48)) & (l_all == 127))
    slot_t = t_all[keep][:N]
    slot_c = c_all[keep][:N]
    slot_l = l_all[keep][:N]

    n_a_slots = int((slot_t < A_TILES).sum())
    a_nodes = np.zeros(N, bool)
    a_nodes[order[:n_a_slots]] = True
    deg_a = np.bincount(dst[a_nodes[src]], minlength=N)

    counts = np.full(TILES, 1024, np.int64)
    counts[23] = counts[48] = 1016
    cum = np.concatenate([[0], np.cumsum(counts)])
    pick = order.copy()
    for band0 in range(0, TILES, 8):
        s0 = int(cum[band0])
        s1 = min(int(cum[min(band0 + 8, TILES)]), N)
        if s0 >= N:
            break
        seg = pick[s0:s1]
        pick[s0:s1] = seg[np.argsort(-deg_a[seg], kind="stable")]

    core_of = np.empty(N, np.int64)
    tile_of = np.empty(N, np.int64)
    lane_of = np.empty(N, np.int64)
    core_of[pick] = slot_c
    tile_of[pick] = slot_t
    lane_of[pick] = slot_l
    return src, dst, core_of, tile_of, lane_of


def preprocess(edge_index):
    src, dst, core_of, tile_of, lane_of = build_assignment(edge_index)
    local_of = tile_of * P + lane_of
    grow = np.where(local_of < A_LOC, core_of * A_LOC + local_of,
                    A_ROWS + core_of * B_LOC + (local_of - A_LOC))
    sg = grow[src]
    bkt = (sg >= A_ROWS).astype(np.int64)
    idxval = (sg - bkt * A_ROWS).astype(np.int64)
    dc = core_of[dst]
    dt_ = tile_of[dst]
    dl = lane_of[dst]

    key = ((dc * TILES + dt_) * 2 + bkt) * P + dl
    ordr = np.argsort(key, kind="stable")
    ks = key[ordr]
    iv = idxval[ordr]
    nkeys = NC * TILES * 2 * P
    cnt = np.bincount(key, minlength=nkeys)
    starts = np.zeros(nkeys + 1, np.int64)
    np.cumsum(cnt, out=starts[1:])
    j = np.arange(len(ks)) - starts[ks]

    cnt4 = cnt.reshape(NC, TILES, 2, P)
    KA = cnt4[:, :, 0, :].max(axis=(0, 2)).astype(np.int64)
    KB = cnt4[:, :, 1, :].max(axis=(0, 2)).astype(np.int64)

    # flat slot streams per (core, sweep): [128 * sum(K)] with per-batch
    # contiguous segments; position = seg_base + (off_t + j)*128 + lane
    def stream_layout(K):
        offt = np.zeros(TILES, np.int64)     # col offset within batch
        segb = np.zeros(NB + 1, np.int64)    # slot base of batch segment
        for b in range(NB):
            o = 0
            for tt in range(TB):
                offt[b * TB + tt] = o
                o += int(K[b * TB + tt])
            segb[b + 1] = segb[b] + P * o
        return offt, segb

    offA, segA = stream_layout(KA)
    offB, segB = stream_layout(KB)
    streams = np.empty((NC, 2), object)
    for c in range(NC):
        streams[c, 0] = np.full(int(segA[NB]), PAD_IDX_A, np.int16)
        streams[c, 1] = np.full(int(segB[NB]), PAD_IDX_B, np.int16)
    kc = ks // (TILES * 2 * P)
    kt = (ks // (2 * P)) % TILES
    kb = (ks // P) % 2
    kl = ks % P
    bb = kt // TB
    offt_of = np.where(kb == 0, offA[kt], offB[kt])
    segb_of = np.where(kb == 0, segA[bb], segB[bb])
    pos = segb_of + (offt_of + j) * P + kl
    for c in range(NC):
        for s in (0, 1):
            m = (kc == c) & (kb == s)
            streams[c, s][pos[m]] = iv[m].astype(np.int16)

    # wrap each (sweep, batch) segment into [16, n/16] and concat cols
    srcw = []
    for c in range(NC):
        parts = []
        for s in (0, 1):
            seg = segA if s == 0 else segB
            for b in range(NB):
                fl = streams[c, s][seg[b]:seg[b + 1]]
                parts.append(fl.reshape(-1, 16).T)
        srcw.append(np.ascontiguousarray(np.concatenate(parts, axis=1)))
    cfg = V2Cfg(KA=tuple(int(k) for k in KA), KB=tuple(int(k) for k in KB))
    return cfg, srcw, core_of, local_of


def make_in_maps(inputs, cfg, srcw, core_of, local_of):
    x = np.asarray(inputs["x"], dtype=np.float32)
    W1 = np.asarray(inputs["W1"], dtype=np.float32)
    a1_src = np.asarray(inputs["a1_src"], dtype=np.float32).reshape(1, D1)
    a1_dst = np.asarray(inputs["a1_dst"], dtype=np.float32).reshape(1, D1)
    b1 = np.asarray(inputs["b1"], dtype=np.float32).reshape(1, D1)
    W2 = np.asarray(inputs["W2"], dtype=np.float32)
    a2_src = np.asarray(inputs["a2_src"], dtype=np.float32).reshape(1, D2)
    a2_dst = np.asarray(inputs["a2_dst"], dtype=np.float32).reshape(1, D2)
    b2 = np.asarray(inputs["b2"], dtype=np.float32).reshape(1, D2)

    w1_dev = np.ascontiguousarray(
        W1.reshape(KC, P, D1).transpose(1, 0, 2).reshape(P, KC * D1)
    ).astype(BF)
    consts = {
        "w1": w1_dev, "w2": W2.astype(BF),
        "a1s": np.broadcast_to(a1_src, (P, D1)).copy(),
        "a1d": np.broadcast_to(a1_dst, (P, D1)).copy(),
        "a2s": np.broadcast_to(a2_src, (P, D2)).copy(),
        "a2d": np.broadcast_to(a2_dst, (P, D2)).copy(),
        "b1r": np.broadcast_to(b1, (P, D1)).copy(),
        "b2r": np.broadcast_to(b2, (P, D2)).copy(),
        "padc": np.full(H1, -30.0, dtype=BF).view(np.int8).reshape(1, 2 * H1),
    }
    xbf = x.astype(BF)
    in_maps = []
    for c in range(NC):
        nodes = np.where(core_of == c)[0]
        xTc = np.zeros((F, SHARD_PAD), dtype=BF)
        xTc[:, local_of[nodes]] = xbf[nodes].T
        in_maps.append({"xT": xTc, "srcW": srcw[c], **consts})
    return in_maps


def assemble_output(results, core_of, local_of):
    outg = np.zeros((N, D2), np.float32)
    for c in range(NC):
        nodes = np.where(core_of == c)[0]
        outg[nodes] = results[c]["out"][local_of[nodes]]
    return outg


# ---------------- public entry point ----------------

_CACHE = {}


def kernel(**inputs) -> np.ndarray:
    ei = np.asarray(inputs["edge_index"]).astype(np.int64)
    cfg, srcw, core_of, local_of = preprocess(ei)
    if cfg not in _CACHE:
        _CACHE[cfg] = build_program(cfg)
    nc = _CACHE[cfg]
    in_maps = make_in_maps(inputs, cfg, srcw, core_of, local_of)
    from concourse import bass_utils
    res = bass_utils.run_bass_kernel_spmd(
        nc, in_maps, core_ids=list(range(NC)))
    return assemble_output(res.results, core_of, local_of)


# ---------------- bench harness hooks ----------------

def bench_build(inputs):
    ei = np.asarray(inputs["edge_index"]).astype(np.int64)
    cfg, srcw, core_of, local_of = preprocess(ei)
    nc = build_program(cfg)
    in_maps = make_in_maps(inputs, cfg, srcw, core_of, local_of)
    return nc, in_maps, (core_of, local_of)


def bench_assemble(outs, out_names, out_avals, n_cores, ctx):
    core_of, local_of = ctx
    i = out_names.index("out")
    arr = np.asarray(outs[i]).reshape(n_cores, *out_avals[i].shape)
    results = [{"out": arr[c]} for c in range(n_cores)]
    return assemble_output(results, core_of, local_of)
